# revision 1
# baseline (speedup 1.0000x reference)
"""GAT 2-layer Trainium2 kernel, v4.

Vs v3 (256us):
  - p transposed by the DMA xbar engine (dma_start_transpose, one call per
    head: [128,512] -> [128,(c,d),128] block layout, verified exact) —
    removes 144 PE transposes + 36 scalar-engine pT copies per core.
  - adjacency mask applied POST-exp as a 0/1 multiply on the vector engine
    (p = adj * max(exp(z), exp(a z)); exactly equals the -9e15 mask since
    masked lanes multiply to 0 before the row sum) — removes the per-head
    ident@mneg PE matmul; z in PSUM is just the e2 rank-1 broadcast
    ([128,256], shared by both node chunks).
  - aug matmuls head-PAIRED (rhs = two heads' embW, free dim 512) to halve
    LDWEIGHTS count (walrus emits one LDW per matmul, no dedup).
  - layer-2 e1/e2 columns via 16 [2,N]-row matmuls (lhsT=woe) instead of 32
    woe-rhs matmuls — no duplicate hT loads, and e2 lands as a row ready
    for the rank-1 broadcast.
"""

import numpy as np
from contextlib import ExitStack

import concourse.bass as bass
import concourse.tile as tile
from concourse import mybir, bacc
from concourse.bass_utils import run_bass_kernel_spmd

f32 = mybir.dt.float32
LOWP = mybir.dt.float16
NPLOW = np.float16
bf16 = mybir.dt.bfloat16
AF = mybir.ActivationFunctionType
AL = mybir.AluOpType

B, N, F, O, H, OUT = 32, 256, 300, 256, 8, 512
VOCAB = 200
NCORES = 8
GPC = B // NCORES
NC = N // 128
VC = 2
KC2 = (H * O) // 128
ALPHA = 0.2


def _build_nc():
    nc = bacc.Bacc("TRN2", target_bir_lowering=False, debug=False,
                   num_devices=NCORES)

    oh_d = nc.dram_tensor("oh", [GPC, 128, VC, N], LOWP, kind="ExternalInput").ap()
    adj_d = nc.dram_tensor("adj01", [GPC, 128, NC, N], LOWP, kind="ExternalInput").ap()
    npm_d = nc.dram_tensor("npm", [GPC, 128, NC], f32, kind="ExternalInput").ap()
    embw_d = nc.dram_tensor("embw", [128, VC, H, O], LOWP, kind="ExternalInput").ap()
    e12w_d = nc.dram_tensor("e12w", [128, VC, 2 * H], LOWP, kind="ExternalInput").ap()
    wo2_d = nc.dram_tensor("wo2", [128, KC2, OUT], bf16, kind="ExternalInput").ap()
    woe_d = nc.dram_tensor("woe", [128, KC2, 2], bf16, kind="ExternalInput").ap()
    identf_d = nc.dram_tensor("identf", [128, 128], f32, kind="ExternalInput").ap()
    out_d = nc.dram_tensor("out", [GPC, 128, NC, OUT], LOWP, kind="ExternalOutput").ap()

    with tile.TileContext(nc) as tc, ExitStack() as ctx:
        const = ctx.enter_context(tc.tile_pool(name="const", bufs=1))
        gpool = ctx.enter_context(tc.tile_pool(name="gpool", bufs=4))
        gl2 = ctx.enter_context(tc.tile_pool(name="gl2", bufs=4))
        hpool = ctx.enter_context(tc.tile_pool(name="hpool", bufs=8))
        hbig = ctx.enter_context(tc.tile_pool(name="hbig", bufs=4))
        # PSUM banks: aug 2 + big 3 + bc 2 + small 1 = 8
        psum = ctx.enter_context(tc.tile_pool(name="psum", bufs=1, space="PSUM"))

        identf = const.tile([128, 128], f32)
        nc.sync.dma_start(identf[:], identf_d)
        ones_b = const.tile([65, 128], LOWP)
        nc.vector.memset(ones_b[:], 1.0)
        embw = const.tile([128, VC, H, O], LOWP)
        nc.sync.dma_start(embw[:].rearrange("p v h o -> p (v h o)"),
                          embw_d.rearrange("p v h o -> p (v h o)"))
        e12w = const.tile([128, VC, 2 * H], LOWP)
        nc.sync.dma_start(e12w[:].rearrange("p v c -> p (v c)"),
                          e12w_d.rearrange("p v c -> p (v c)"))
        wo2 = const.tile([128, KC2, OUT], bf16)
        nc.sync.dma_start(wo2[:].rearrange("p k o -> p (k o)"),
                          wo2_d.rearrange("p k o -> p (k o)"))
        woe = const.tile([128, KC2, 2], bf16)
        nc.sync.dma_start(woe[:].rearrange("p k e -> p (k e)"),
                          woe_d.rearrange("p k e -> p (k e)"))

        G = {}

        def emit_setup(g):
            s = G[g] = {}
            oh_sb = gpool.tile([128, VC, N], LOWP)
            nc.sync.dma_start(oh_sb[:].rearrange("p v n -> p (v n)"),
                              oh_d[g].rearrange("p v n -> p (v n)"))
            adj01 = gpool.tile([128, NC, N], LOWP)
            nc.sync.dma_start(adj01[:].rearrange("p c n -> p (c n)"),
                              adj_d[g].rearrange("p c n -> p (c n)"))
            npm = gpool.tile([128, NC], f32)
            nc.sync.dma_start(npm[:], npm_d[g])

            e12_ps = psum.tile([2 * H, N], f32, tag="small", name="e12ps", bufs=1)
            for vc in range(VC):
                nc.tensor.matmul(e12_ps[:], lhsT=e12w[:, vc, :],
                                 rhs=oh_sb[:, vc, :],
                                 start=(vc == 0), stop=(vc == VC - 1))
            e12_f = gpool.tile([2 * H, N], f32)
            nc.scalar.copy(e12_f[:], e12_ps[:])
            e2rs_f = gpool.tile([65, 3, N], f32)
            for i in range(3):
                nh = min(3, H - 3 * i)
                nc.scalar.dma_start(e2rs_f[32 * i:32 * i + 1, 0:nh, :],
                                    e12_f[H + 3 * i:H + 3 * i + nh, :])
            e2rs = gpool.tile([65, 3, N], LOWP)
            nc.vector.tensor_copy(e2rs[:].rearrange("p a n -> p (a n)"),
                                  e2rs_f[:].rearrange("p a n -> p (a n)"))
            e1cols = gpool.tile([128, NC, H], f32)
            for c in range(NC):
                e1c_ps = psum.tile([128, H], f32, tag="small", name="e1c", bufs=1)
                nc.tensor.transpose(e1c_ps[:],
                                    e12_f[0:H, c * 128:(c + 1) * 128],
                                    identf[0:H, 0:H])
                nc.scalar.copy(e1cols[:, c, :], e1c_ps[:])
            e1a = gpool.tile([128, NC, H], f32)
            nc.vector.tensor_scalar_mul(
                e1a[:].rearrange("p c h -> p (c h)"),
                e1cols[:].rearrange("p c h -> p (c h)"), ALPHA)
            hT = hbig.tile([128, KC2, N], bf16)
            s.update(oh_sb=oh_sb, adj01=adj01, npm=npm, e2rs=e2rs,
                     e1cols=e1cols, e1a=e1a, hT=hT, wh={})

        def emit_aug(g, h0):
            # head-paired aug: rhs covers heads h0,h0+1 (free dim 512)
            s = G[g]
            ps = {c: psum.tile([128, 2, O], f32, tag="aug",
                               name=f"aug{c}", bufs=2) for c in range(NC)}
            for c in range(NC):
                for vc in range(VC):
                    nc.tensor.matmul(
                        ps[c][:].rearrange("p h o -> p (h o)"),
                        lhsT=s["oh_sb"][:, vc, c * 128:(c + 1) * 128],
                        rhs=embw[:, vc, h0:h0 + 2, :].rearrange(
                            "p h o -> p (h o)"),
                        start=(vc == 0), stop=(vc == VC - 1))
            for i, h in enumerate((h0, h0 + 1)):
                wh_sb = hpool.tile([128, NC, O], LOWP, tag="wh")
                for c in range(NC):
                    nc.scalar.copy(wh_sb[:, c, :], ps[c][:, i, :])
                s["wh"][h] = wh_sb

        def softmax_p(e2bc, e1c, e1ac, adj01, tag):
            """masked unnormalized p + row sums; z = e2bc (PSUM) + e1 bias."""
            eA = hpool.tile([128, NC, N], LOWP, tag=f"eA{tag}", name="eA")
            eB = hpool.tile([128, NC, N], LOWP, tag=f"eB{tag}", name="eB")
            for c in range(NC):
                nc.scalar.activation(eA[:, c, :], e2bc, AF.Exp,
                                     bias=e1c[:, c:c + 1])
                nc.scalar.activation(eB[:, c, :], e2bc, AF.Exp,
                                     bias=e1ac[:, c:c + 1], scale=ALPHA)
            p_sb = hpool.tile([128, NC, N], LOWP, tag=f"p{tag}", name="p_sb")
            nc.vector.tensor_tensor(
                p_sb[:].rearrange("p c n -> p (c n)"),
                eA[:].rearrange("p c n -> p (c n)"),
                eB[:].rearrange("p c n -> p (c n)"), op=AL.max)
            nc.vector.tensor_tensor(
                p_sb[:].rearrange("p c n -> p (c n)"),
                p_sb[:].rearrange("p c n -> p (c n)"),
                adj01[:].rearrange("p c n -> p (c n)"), op=AL.mult)
            zsum = hpool.tile([128, NC], f32, tag=f"zs{tag}", name="zsum")
            for c in range(NC):
                nc.vector.tensor_scalar(p_sb[:, c, :], p_sb[:, c, :],
                                        1.0, 0.0, op0=AL.mult, op1=AL.add,
                                        accum_out=zsum[:, c:c + 1])
            return p_sb, zsum

        def emit_head(g, h):
            s = G[g]
            wh_sb = s["wh"].pop(h)
            hT = s["hT"]
            gi, gj = h // 3, h % 3
            e2t = psum.tile([128, N], f32, tag="bc", name="e2bc", bufs=2)
            nc.tensor.matmul(e2t[:], lhsT=ones_b[32 * gi:32 * gi + 1, :],
                             rhs=s["e2rs"][32 * gi:32 * gi + 1, gj, :],
                             start=True, stop=True)
            e2bc = e2t[:]
            p_sb, zsum = softmax_p(e2bc, s["e1cols"][:, :, h],
                                   s["e1a"][:, :, h], s["adj01"], "1")
            zinv = hpool.tile([128, NC], f32, tag="zi")
            nc.vector.reciprocal(zinv[:], zsum[:])
            for c in range(NC):
                nc.vector.tensor_scalar_mul(
                    p_sb[:, c, :], p_sb[:, c, :], zinv[:, c:c + 1])
            # DMA xbar transpose: pT[m',(c,d),u] = p[c*128+u, d*128+m']
            pT = hpool.tile([128, NC * NC, 128], LOWP, tag="pT")
            nc.sync.dma_start_transpose(
                pT[:], p_sb[:].rearrange("p c n -> p (c n)"))
            # pT block j=(c,d): rows m'@chunk d, cols node chunk c
            pTv = pT[:].rearrange("p (c d) u -> p d c u", d=NC)
            ops = psum.tile([128, NC, N], f32, tag="big", name="o1ps", bufs=3)
            for oc in range(NC):
                for mc in range(NC):
                    nc.tensor.matmul(
                        ops[:, oc, :],
                        lhsT=wh_sb[:, mc, oc * 128:(oc + 1) * 128],
                        rhs=pTv[:, mc],
                        start=(mc == 0), stop=(mc == NC - 1))
            # hT = elu(x) = min(relu(x), exp(x)-1)  (exact; min() makes
            # exp-overflow inf harmless; bf16 keeps small elu precise)
            at = hpool.tile([128, NC, N], bf16, tag="at")
            nc.scalar.activation(at[:].rearrange("p c n -> p (c n)"),
                                 ops[:].rearrange("p c n -> p (c n)"), AF.Exp)
            nc.vector.tensor_scalar(
                at[:].rearrange("p c n -> p (c n)"),
                at[:].rearrange("p c n -> p (c n)"),
                1.0, None, op0=AL.subtract)
            hs = hT[:, h * NC:(h + 1) * NC, :].rearrange("p c n -> p (c n)")
            nc.vector.tensor_scalar(
                hs, ops[:].rearrange("p c n -> p (c n)"),
                0.0, None, op0=AL.max)
            nc.vector.tensor_tensor(
                hs, at[:].rearrange("p c n -> p (c n)"), hs, op=AL.min)

        def emit_l2(g):
            s = G[g]
            npm, hT = s["npm"], s["hT"]
            wh2_sb = gl2.tile([128, NC, OUT], bf16)
            for c in range(NC):
                wps = psum.tile([128, OUT], f32, tag="big", name="wh2ps", bufs=3)
                for k in range(KC2):
                    nc.tensor.matmul(wps[:],
                                     lhsT=hT[:, k, c * 128:(c + 1) * 128],
                                     rhs=wo2[:, k, :],
                                     start=(k == 0), stop=(k == KC2 - 1))
                nc.scalar.activation(wh2_sb[:, c, :], wps[:], AF.Copy,
                                     scale=npm[:, c:c + 1])
            # e1/e2 rows: [2, N] = woe.T @ hT (+ -colsum correction)
            er_ps = psum.tile([2, N], f32, tag="small", name="erps", bufs=1)
            for k in range(KC2):
                nc.tensor.matmul(er_ps[:], lhsT=woe[:, k, :],
                                 rhs=hT[:, k, :],
                                 start=(k == 0), stop=(k == KC2 - 1))
            er_f = gl2.tile([2, N], f32)
            nc.scalar.copy(er_f[:], er_ps[:])  # npm==1 rows (spec: fill ones)
            e2r_f = gl2.tile([1, N], f32)
            nc.scalar.dma_start(e2r_f[:], er_f[1:2, :])
            e2row = gl2.tile([1, N], LOWP)
            nc.vector.tensor_copy(e2row[:], e2r_f[:])
            # e1 column per chunk + alpha copy
            e1col2 = gl2.tile([128, NC, 1], f32)
            for c in range(NC):
                ec_ps = psum.tile([128, 1], f32, tag="small", name="ec2", bufs=1)
                nc.tensor.transpose(ec_ps[:],
                                    er_f[0:1, c * 128:(c + 1) * 128],
                                    identf[0:1, 0:1])
                nc.scalar.copy(e1col2[:, c, :], ec_ps[:])
            e1a2 = gl2.tile([128, NC, 1], f32)
            nc.vector.tensor_scalar_mul(
                e1a2[:].rearrange("p c e -> p (c e)"),
                e1col2[:].rearrange("p c e -> p (c e)"), ALPHA)
            e2bc = psum.tile([128, N], f32, tag="bc", name="e2bc2", bufs=2)
            nc.tensor.matmul(e2bc[:], lhsT=ones_b[0:1, :], rhs=e2row[:],
                             start=True, stop=True)
            p2, z2sum = softmax_p(e2bc[:], e1col2[:, :, 0], e1a2[:, :, 0],
                                  s["adj01"], "1")
            z2inv = hpool.tile([128, NC], f32, tag="zi")
            nc.vector.reciprocal(z2inv[:], z2sum[:])
            sc2 = gl2.tile([128, NC], f32)
            nc.vector.tensor_mul(sc2[:], z2inv[:], npm[:])
            pT2 = hpool.tile([128, NC * NC, 128], LOWP, tag="pT")
            nc.sync.dma_start_transpose(
                pT2[:], p2[:].rearrange("p c n -> p (c n)"))
            out_sb = gl2.tile([128, NC, OUT], LOWP)
            a2 = gl2.tile([128, NC, OUT], LOWP)
            for c in range(NC):
                o2ps = psum.tile([128, OUT], f32, tag="big", name="o2ps", bufs=3)
                for mc in range(NC):
                    nc.tensor.matmul(
                        o2ps[:], lhsT=pT2[:, c * NC + mc, :],
                        rhs=wh2_sb[:, mc, :],
                        start=(mc == 0), stop=(mc == NC - 1))
                nc.scalar.activation(a2[:, c, :], o2ps[:], AF.Exp,
                                     scale=sc2[:, c:c + 1])
                nc.vector.tensor_scalar(out_sb[:, c, :], o2ps[:],
                                        sc2[:, c:c + 1], 0.0,
                                        op0=AL.mult, op1=AL.max)
            a2m = gl2.tile([128, NC, OUT], LOWP)
            nc.vector.tensor_scalar(
                a2m[:].rearrange("p c o -> p (c o)"),
                a2[:].rearrange("p c o -> p (c o)"),
                1.0, 0.0, op0=AL.subtract, op1=AL.min)
            nc.vector.tensor_tensor(
                out_sb[:].rearrange("p c o -> p (c o)"),
                a2m[:].rearrange("p c o -> p (c o)"),
                out_sb[:].rearrange("p c o -> p (c o)"), op=AL.add)
            nc.gpsimd.dma_start(out_d[g].rearrange("p c o -> p (c o)"),
                                out_sb[:].rearrange("p c o -> p (c o)"))
            del G[g]

        gs = list(range(GPC))
        for g in gs:
            emit_setup(g)
        for g in gs:
            emit_aug(g, 0)
        for h in range(H):
            for g in gs:
                emit_head(g, h)
            if h % 2 == 1 and h + 1 < H:
                for g in gs:
                    emit_aug(g, h + 1)
        for g in gs:
            emit_l2(g)

    nc.compile()
    return nc


_NC_CACHE = {}


def build_kernel():
    if "v9" not in _NC_CACHE:
        _NC_CACHE["v9"] = _build_nc()
    return _NC_CACHE["v9"]


def _host_prep(fea, adj, non_pad_mask, embed, W_heads, a_heads, W_out, a_out):
    W64 = W_heads.astype(np.float64)
    w1 = np.einsum("hfo,ho->hf", W64, a_heads[:, :O].astype(np.float64))
    w2 = np.einsum("hfo,ho->hf", W64, a_heads[:, O:].astype(np.float64))
    emb64 = np.zeros((VC * 128, F))
    emb64[:VOCAB] = embed.astype(np.float64)
    embw = np.einsum("vf,hfo->hvo", emb64, W64)          # (H, 256, O)
    embw = np.ascontiguousarray(                          # [128, VC, H, O]
        embw.reshape(H, VC, 128, O).transpose(2, 1, 0, 3))
    e12 = emb64 @ np.concatenate([w1.T, w2.T], axis=1)
    e12w = np.ascontiguousarray(
        e12.reshape(VC, 128, 2 * H).transpose(1, 0, 2))

    Wo64 = W_out.astype(np.float64)
    w1o = Wo64 @ a_out[:OUT].astype(np.float64)
    w2o = Wo64 @ a_out[OUT:].astype(np.float64)
    wo2 = np.ascontiguousarray(
        Wo64.reshape(KC2, 128, OUT).transpose(1, 0, 2))
    woe = np.ascontiguousarray(
        np.stack([w1o, w2o], axis=1).reshape(KC2, 128, 2).transpose(1, 0, 2))

    vidx = np.arange(VC * 128).reshape(VC, 128)
    oh = (fea[:, None, None, :] == vidx[None, :, :, None])
    oh = np.ascontiguousarray(oh.transpose(0, 2, 1, 3)).astype(np.float64)

    adj01 = np.ascontiguousarray(
        adj.astype(np.float64).reshape(B, NC, 128, N).transpose(0, 2, 1, 3))
    npm = np.ascontiguousarray(
        non_pad_mask.reshape(B, NC, 128).transpose(0, 2, 1)).astype(np.float32)

    import ml_dtypes
    bf = ml_dtypes.bfloat16
    lp = NPLOW
    return (oh.astype(lp), adj01.astype(lp), npm, embw.astype(lp),
            e12w.astype(lp), wo2.astype(bf), woe.astype(bf))


def kernel(fea, adj, non_pad_mask, embed, W_heads, a_heads, W_out, a_out,
           _mm_dt=None, _trace=False):
    oh, adj01, npm, embw, e12w, wo2, woe = _host_prep(
        fea, adj, non_pad_mask, embed, W_heads, a_heads, W_out, a_out)

    nc = build_kernel()
    identf = np.eye(128, dtype=np.float32)
    in_maps = []
    for i in range(NCORES):
        sl = slice(i * GPC, (i + 1) * GPC)
        in_maps.append({
            "oh": oh[sl], "adj01": adj01[sl], "npm": npm[sl],
            "embw": embw, "e12w": e12w, "wo2": wo2, "woe": woe,
            "identf": identf,
        })
    res = run_bass_kernel_spmd(nc, in_maps, core_ids=list(range(NCORES)),
                               trace=_trace)
    outs = []
    for i in range(NCORES):
        o = np.asarray(res.results[i]["out"]).astype(np.float32)
        outs.append(o.transpose(0, 2, 1, 3).reshape(GPC, N, OUT))
    full = np.concatenate(outs, axis=0)
    if _trace:
        kernel.last_results = res
    return full

